# revision 1
# baseline (speedup 1.0000x reference)
"""MoE block (router + top-2 expert MLPs) on 8 Trainium2 NeuronCores.

Strategy (expert-parallel):
  - Router (x @ Wr + br, top-2, softmax) computed on host with jax using the
    exact expression of the reference so expert selection matches bitwise.
  - Tokens are dispatched by expert: core e receives the tokens whose top-2
    includes expert e (padded to a fixed capacity CAP), plus expert e's
    weights W1[e]/b1[e]/W2[e]/b2[e].
  - Each core runs a Bass/Tile kernel computing
        y = sigmoid(relu(x @ W1 + b1) @ W2 + b2)
    for its CAP tokens with fp16 matmuls (fp32 PSUM accumulation; fp16
    streams 2 elements per 4-byte SBUF read so the PE runs at 1 col/cycle
    vs 2 cycles/col for fp32/fp32r operands).
  - Host combines: out[t] = sum_k weight[t,k] * y_e[t].

Kernel layout per core:
  xT [D, CAP] fp16 (tokens gathered+transposed+converted on host),
  W1 [D, H] fp16, W2 [H, D] fp16, b1 fp32, b2 fp16.
  All 32 weight h-chunks are SBUF-resident (fp16 halves the footprint), so
  weights stream from HBM exactly once. Loop over 3 token groups of 384;
  per group y accumulates in PSUM (3 x [128 tok, 1024 d] fp32 tiles =
  6 banks) across all 32 h-chunks; the h tile (128 h x 384 tok) uses 2
  more banks. Layer 1: lhsT=W1 chunk, rhs=xT chunk -> h^T in PSUM;
  relu+b1 via ScalarE into fp16 SBUF; layer 2: lhsT=h tile slice,
  rhs=W2 chunk, accumulating into the y PSUM tiles. b2 is pre-added via a
  rank-1 (K=1) matmul with a ones vector; sigmoid+store per group.
"""

import numpy as np

D = 1024
H = 4096
E = 8
TOPK = 2
B = 4096

P = 128
KC = D // P          # 8 contraction chunks for layer 1
HC = H // P          # 32 h chunks
GROUP = 384          # tokens per PSUM-resident group
MSUB = GROUP // P    # 3 token subtiles per group
NGRP = 3             # groups per core
CAP = GROUP * NGRP   # 1152 token capacity per core
N_CORES = 8

_compiled_nc = None


def _build_nc(mm_dtype_name: str = "float16"):
    import concourse.bacc as bacc
    import concourse.mybir as mybir
    import concourse.tile as tile

    f32 = mybir.dt.float32
    mmdt = getattr(mybir.dt, mm_dtype_name)
    AF = mybir.ActivationFunctionType

    nc = bacc.Bacc("TRN2", target_bir_lowering=False, debug=False,
                   enable_asserts=False)

    # Host-prearranged layouts: every chunk is one contiguous DMA.
    #   xt[g, kc, p, t'] = x_tokens[g*GROUP + t', kc*128 + p]
    #   w1[hc, p, kc, h'] = W1[kc*128 + p, hc*128 + h']
    xt_d = nc.dram_tensor("xt", (NGRP, KC, P, GROUP), mmdt,
                          kind="ExternalInput")
    w1_d = nc.dram_tensor("w1", (HC, P, KC, P), mmdt, kind="ExternalInput")
    b1_d = nc.dram_tensor("b1", (H,), f32, kind="ExternalInput")
    w2_d = nc.dram_tensor("w2", (H, D), mmdt, kind="ExternalInput")
    b2_d = nc.dram_tensor("b2", (D,), f32, kind="ExternalInput")
    b2h_d = nc.dram_tensor("b2h", (D,), mmdt, kind="ExternalInput")
    ones_d = nc.dram_tensor("ones", (P,), mmdt, kind="ExternalInput")
    y_d = nc.dram_tensor("y", (CAP, D), f32, kind="ExternalOutput")

    # DRAM views with the partition dim (128) first.
    w2_v = w2_d.ap().rearrange("(hc p) d -> p hc d", p=P)      # [128, 32, D]
    b1_v = b1_d.ap().rearrange("(hc p) -> p hc", p=P)          # [128, 32]
    y_v = y_d.ap().rearrange("(g m p) d -> g m p d", g=NGRP, m=MSUB)

    with tile.TileContext(nc) as tc:
        with (
            tc.tile_pool(name="const", bufs=1) as cpool,
            tc.tile_pool(name="wres", bufs=1) as respool,
            tc.tile_pool(name="hsb", bufs=2) as hpool,
            tc.tile_pool(name="yout", bufs=2) as ypool_sb,
            tc.tile_pool(name="hps", bufs=2, space="PSUM") as hpsum,
            tc.tile_pool(name="yps", bufs=1, space="PSUM") as ypsum,
        ):
            # x + weights; ordered so the PE can start ASAP: x(g0,kc0) +
            # first weight chunk -> rest of x(g0) -> b1 (first relu) ->
            # weight stream with x(g1)/x(g2) and b2 interleaved.
            x_sb = [cpool.tile([P, KC, GROUP], mmdt, name=f"x{g}",
                               tag=f"x{g}") for g in range(NGRP)]
            # PE warm-up: dependency-free matmuls on an uninitialized
            # scratch tile get the PE past the HAM half-clock window while
            # the first input DMAs are still in flight. Results land in a
            # scratch PSUM tile and are never read.
            scratch_sb = cpool.tile([P, GROUP], mmdt)
            nc.vector.memset(scratch_sb[:], 0.0)
            warm_ps = hpsum.tile([P, GROUP], f32, name="warm_ps", tag="hps")
            for _ in range(14):
                nc.tensor.matmul(warm_ps[:], scratch_sb[:, :P],
                                 scratch_sb[:], start=True, stop=True)
            nc.sync.dma_start(x_sb[0][:, 0, :], xt_d.ap()[0, 0])

            w1_all = respool.tile([P, HC, KC, P], mmdt)
            w2_all = respool.tile([P, HC, D], mmdt)
            for hc in range(HC):
                nc.sync.dma_start(w1_all[:, hc], w1_d.ap()[hc])
                nc.sync.dma_start(w2_all[:, hc, :], w2_v[:, hc, :])
                if hc == 0:
                    for kc in range(1, 4):
                        nc.sync.dma_start(x_sb[0][:, kc, :], xt_d.ap()[0, kc])
                if hc == 1:
                    for kc in range(4, KC):
                        nc.sync.dma_start(x_sb[0][:, kc, :], xt_d.ap()[0, kc])
                    b1_sb = cpool.tile([P, HC], f32)
                    nc.sync.dma_start(b1_sb[:], b1_v)
                    b2h_sb = cpool.tile([1, D], mmdt)
                    nc.sync.dma_start(b2h_sb[:], b2h_d.ap()[None, :])
                    ones_sb = cpool.tile([1, P], mmdt)
                    nc.sync.dma_start(ones_sb[:], ones_d.ap()[None, :])
                if hc == 2:
                    for kc in range(KC):
                        nc.sync.dma_start(x_sb[1][:, kc, :], xt_d.ap()[1, kc])
                    b2_full = cpool.tile([P, D], f32)
                    nc.sync.dma_start(
                        b2_full[:],
                        b2_d.ap()[None, :].broadcast_to([P, D]))
                if hc == 5:
                    for kc in range(KC):
                        nc.sync.dma_start(x_sb[2][:, kc, :], xt_d.ap()[2, kc])

            for g in range(NGRP):
                yps = [ypsum.tile([P, D], f32, name=f"yps{m}", tag=f"yps{m}")
                       for m in range(MSUB)]

                last = g == NGRP - 1
                if last:
                    # rank-1 b2 matmuls keep the last group's tail short
                    # (no DVE add on the critical path)
                    for m in range(MSUB):
                        for h2 in range(2):
                            nc.tensor.matmul(
                                yps[m][:, h2 * 512:(h2 + 1) * 512],
                                ones_sb[:],
                                b2h_sb[:, h2 * 512:(h2 + 1) * 512],
                                start=True, stop=False,
                            )

                for hc in range(HC):
                    w1c = w1_all[:, hc]
                    w2c = w2_all[:, hc, :]

                    # Layer 1: h^T chunk [128 h, GROUP tok]
                    hps = hpsum.tile([P, GROUP], f32)
                    for kc in range(KC):
                        nc.tensor.matmul(
                            hps[:],
                            w1c[:, kc, :],
                            x_sb[g][:, kc, :],
                            start=(kc == 0), stop=(kc == KC - 1),
                        )
                    hsb = hpool.tile([P, GROUP], mmdt)
                    nc.scalar.activation(
                        hsb[:], hps[:], AF.Relu, bias=b1_sb[:, hc:hc + 1])

                    # Layer 2: accumulate into y PSUM
                    for m in range(MSUB):
                        lhs = hsb[:, m * P:(m + 1) * P]
                        for h2 in range(2):
                            nc.tensor.matmul(
                                yps[m][:, h2 * 512:(h2 + 1) * 512],
                                lhs,
                                w2c[:, h2 * 512:(h2 + 1) * 512],
                                start=(hc == 0 and not last),
                                stop=(hc == HC - 1),
                            )

                # Epilogue: (+ b2 via DVE unless folded), sigmoid, store
                for m in range(MSUB):
                    if not last:
                        nc.vector.tensor_add(yps[m][:], yps[m][:], b2_full[:])
                    yo = ypool_sb.tile([P, D], f32)
                    nc.scalar.activation(yo[:], yps[m][:], AF.Sigmoid)
                    nc.sync.dma_start(y_v[g, m], yo[:])

    nc.compile()
    return nc


def _routing(x, Wr, br):
    """Router computed with the same jax expression as the reference."""
    import jax
    import jax.numpy as jnp

    logits = jnp.asarray(x) @ jnp.asarray(Wr) + jnp.asarray(br)
    topk_vals, topk_idx = jax.lax.top_k(logits, TOPK)
    weights = jax.nn.softmax(topk_vals, axis=-1)
    return np.asarray(topk_idx), np.asarray(weights, np.float32)


def _get_nc():
    global _compiled_nc
    if _compiled_nc is None:
        _compiled_nc = _build_nc()
    return _compiled_nc


def kernel(x, Wr, br, W1, b1, W2, b2, _trace=False, _trace_kwargs=None):
    from concourse import bass_utils

    x = np.ascontiguousarray(np.asarray(x, dtype=np.float32))
    Wr = np.asarray(Wr, dtype=np.float32)
    br = np.asarray(br, dtype=np.float32)
    W1 = np.asarray(W1, dtype=np.float32)
    b1 = np.asarray(b1, dtype=np.float32)
    W2 = np.asarray(W2, dtype=np.float32)
    b2 = np.asarray(b2, dtype=np.float32)

    topk_idx, wts = _routing(x, Wr, br)

    # Per-expert token lists and weights
    tok_lists = []
    wt_lists = []
    for e in range(E):
        mask = topk_idx == e                      # [B, TOPK]
        toks = np.nonzero(mask.any(axis=1))[0]
        # weight of expert e for each selected token (exactly one slot matches)
        slot = mask[toks].argmax(axis=1)
        tok_lists.append(toks)
        wt_lists.append(wts[toks, slot])

    nc = _get_nc()

    xh = x.astype(np.float16)
    W1h = W1.astype(np.float16)
    # chunk-major w1 layout: [HC, P, KC, P]
    W1ch = [np.ascontiguousarray(
        W1h[e].reshape(KC, P, HC, P).transpose(2, 1, 0, 3)) for e in range(E)]
    W2h = W2.astype(np.float16)

    out = np.zeros((B, D), dtype=np.float32)
    max_count = max(len(t) for t in tok_lists)
    n_waves = max(1, -(-max_count // CAP))
    last_result = None
    for wave in range(n_waves):
        in_maps = []
        for e in range(E):
            toks = tok_lists[e][wave * CAP:(wave + 1) * CAP]
            xpad = np.zeros((CAP, D), dtype=np.float16)
            if len(toks):
                xpad[:len(toks)] = xh[toks]
            xt = np.ascontiguousarray(
                xpad.reshape(NGRP, GROUP, KC, P).transpose(0, 2, 3, 1))
            in_maps.append({
                "xt": xt,
                "ones": np.ones((P,), dtype=np.float16),
                "b2h": b2[e].astype(np.float16),
                "w1": W1ch[e],
                "b1": np.ascontiguousarray(b1[e]),
                "w2": np.ascontiguousarray(W2h[e]),
                "b2": np.ascontiguousarray(b2[e]),
            })
        res = bass_utils.run_bass_kernel_spmd(
            nc, in_maps, core_ids=list(range(N_CORES)),
            trace=_trace, **(_trace_kwargs or {}))
        last_result = res
        for e in range(E):
            toks = tok_lists[e][wave * CAP:(wave + 1) * CAP]
            if len(toks) == 0:
                continue
            y_e = res.results[e]["y"][:len(toks)]
            out[toks] += wt_lists[e][wave * CAP:(wave + 1) * CAP][:, None] * y_e

    if _trace:
        kernel.last_result = last_result
    return out



# revision 2
# speedup vs baseline: 1.9934x; 1.9934x over previous
"""MoE block (router + top-2 expert MLPs) on 8 Trainium2 NeuronCores.

Strategy (expert-parallel, fp8 DoubleRow):
  - Router (x @ Wr + br, top-2, softmax) computed on host with jax using the
    exact expression of the reference so expert selection matches bitwise.
  - Tokens are dispatched by expert: core e receives the tokens whose top-2
    includes expert e (padded to a fixed capacity CAP), plus expert e's
    weights W1[e]/b1[e]/W2[e]/b2[e].
  - Each core runs a Bass/Tile kernel computing
        y = sigmoid(relu(x @ W1 + b1) @ W2 + b2)
    for its CAP tokens with fp8-e4m3 matmuls in DoubleRow perf mode
    (2 fp8 weights per PE cell -> K=256 contraction per pass, ~1.4-1.8x
    the fp16 matmul throughput). fp32 PSUM accumulation.
  - Quantization scales (powers of 2, exact to undo): x*16, W1*2048,
    h*32, W2*4096. relu is positively homogeneous so the h scale folds
    into the layer-1 activation (scale=2^-10 on PSUM, bias=32*b1);
    the final sigmoid applies scale=2^-17 to undo h/W2 scaling. b2 is
    pre-scaled by 2^17 on host (fp32/fp16, exact enough).
  - Host combines: out[t] = sum_k weight[t,k] * y_e[t].

Kernel layout per core:
  xT [NGRP, KC, 128, GROUP] fp8 (tokens gathered+transposed+scaled on host),
  W1 [HC, 128, KC, 128] fp8, W2 [H, D] fp8, b1 fp32 (*32), b2 fp32 (*2^17).
  All weights are SBUF-resident; they stream from HBM exactly once.
  Loop over 3 token groups of 384; per group y accumulates in PSUM
  (3 x [128 tok, 1024 d] fp32 tiles = 6 banks) across 16 h-chunk PAIRS;
  the h PSUM tiles (128 h x 384 tok, 2 banks) double-buffer.
  Layer 1 (per h-chunk): 4 DoubleRow matmuls lhsT=W1[:, kc:kc+2, :],
  rhs=xT[:, kc:kc+2, :] -> h^T in PSUM; relu+b1 via ScalarE into a
  [128, 2, GROUP] fp8 pair tile; layer 2: lhsT=pair tile slice
  [128, 2, 128 tok], rhs=W2 pair [128, 2, 512], DoubleRow, accumulating
  into the y PSUM tiles. The layer-1 work for pair j+2 is issued before
  layer-2 of pair j so the PE never stalls on the relu latency.
  b2 is pre-added via a rank-1 (K=1) fp16 matmul with a ones vector on
  the last group (keeps the tail short); DVE adds it on earlier groups.
"""

import numpy as np

D = 1024
H = 4096
E = 8
TOPK = 2
B = 4096

P = 128
KC = D // P          # 8 contraction chunks for layer 1
HC = H // P          # 32 h chunks
HPAIR = HC // 2      # 16 h-chunk pairs (DoubleRow)
GROUP = 384          # tokens per PSUM-resident group
MSUB = GROUP // P    # 3 token subtiles per group
NGRP = 3             # groups per core
CAP = GROUP * NGRP   # 1152 token capacity per core
N_CORES = 8

# fp8 quantization scales (powers of two; exactly undone on device)
SX = 16.0
S1 = 2048.0
SH = 32.0
S2 = 4096.0

_compiled_nc = None


def _build_nc():
    import concourse.bacc as bacc
    import concourse.mybir as mybir
    import concourse.tile as tile

    f32 = mybir.dt.float32
    f16 = mybir.dt.float16
    fp8 = mybir.dt.float8e4
    AF = mybir.ActivationFunctionType
    DR = mybir.MatmulPerfMode.DoubleRow

    nc = bacc.Bacc("TRN2", target_bir_lowering=False, debug=False,
                   enable_asserts=False)

    # Host-prearranged layouts: every chunk is one contiguous DMA.
    #   xt[g, kc, p, t'] = SX * x_tokens[g*GROUP + t', kc*128 + p]
    #   w1[hc, p, kc, h'] = S1 * W1[kc*128 + p, hc*128 + h']
    xt_d = nc.dram_tensor("xt", (NGRP, KC, P, GROUP), fp8,
                          kind="ExternalInput")
    w1_d = nc.dram_tensor("w1", (HC, P, KC, P), fp8, kind="ExternalInput")
    b1_d = nc.dram_tensor("b1", (H,), f32, kind="ExternalInput")  # *SH
    w2_d = nc.dram_tensor("w2", (H, D), fp8, kind="ExternalInput")  # *S2
    b2_d = nc.dram_tensor("b2", (D,), f32, kind="ExternalInput")  # *SH*S2
    b2h_d = nc.dram_tensor("b2h", (D,), f16, kind="ExternalInput")  # *SH*S2
    ones_d = nc.dram_tensor("ones", (P,), f16, kind="ExternalInput")
    y_d = nc.dram_tensor("y", (CAP, D), f32, kind="ExternalOutput")

    # DRAM views with the partition dim (128) first.
    w2_v = w2_d.ap().rearrange("(hc p) d -> p hc d", p=P)      # [128, 32, D]
    b1_v = b1_d.ap().rearrange("(hc p) -> p hc", p=P)          # [128, 32]
    y_v = y_d.ap().rearrange("(g m p) d -> g m p d", g=NGRP, m=MSUB)

    with tile.TileContext(nc) as tc:
        with (
            tc.tile_pool(name="const", bufs=1) as cpool,
            tc.tile_pool(name="wres", bufs=1) as respool,
            tc.tile_pool(name="hsb", bufs=3) as hpool,
            tc.tile_pool(name="yout", bufs=3) as ypool_sb,
            tc.tile_pool(name="hps", bufs=2, space="PSUM") as hpsum,
            tc.tile_pool(name="yps", bufs=1, space="PSUM") as ypsum,
        ):
            # x + weights; ordered so the PE can start ASAP: x(g0,kc0) +
            # first weight chunk -> rest of x(g0) -> b1 (first relu) ->
            # weight stream with x(g1)/x(g2) and b2 interleaved.
            x_sb = [cpool.tile([P, KC, GROUP], fp8, name=f"x{g}",
                               tag=f"x{g}") for g in range(NGRP)]
            # PE warm-up: dependency-free matmuls on an uninitialized
            # scratch tile get the PE past the HAM half-clock window while
            # the first input DMAs are still in flight. Results land in a
            # scratch PSUM tile and are never read.
            scratch_sb = cpool.tile([P, GROUP], fp8)
            nc.vector.memset(scratch_sb[:], 0.0)
            warm_ps = hpsum.tile([P, GROUP], f32, name="warm_ps", tag="hps")
            for _ in range(14):
                nc.tensor.matmul(warm_ps[:], scratch_sb[:, :P],
                                 scratch_sb[:], start=True, stop=True)
            nc.sync.dma_start(x_sb[0][:, 0, :], xt_d.ap()[0, 0])

            w1_all = respool.tile([P, HC, KC, P], fp8)
            w2_all = respool.tile([P, HC, D], fp8)
            for hc in range(HC):
                nc.sync.dma_start(w1_all[:, hc], w1_d.ap()[hc])
                nc.sync.dma_start(w2_all[:, hc, :], w2_v[:, hc, :])
                if hc == 0:
                    for kc in range(1, 4):
                        nc.sync.dma_start(x_sb[0][:, kc, :], xt_d.ap()[0, kc])
                if hc == 1:
                    for kc in range(4, KC):
                        nc.sync.dma_start(x_sb[0][:, kc, :], xt_d.ap()[0, kc])
                    b1_sb = cpool.tile([P, HC], f32)
                    nc.sync.dma_start(b1_sb[:], b1_v)
                    b2h_sb = cpool.tile([1, D], f16)
                    nc.sync.dma_start(b2h_sb[:], b2h_d.ap()[None, :])
                    ones_sb = cpool.tile([1, P], f16)
                    nc.sync.dma_start(ones_sb[:], ones_d.ap()[None, :])
                if hc == 2:
                    for kc in range(KC):
                        nc.sync.dma_start(x_sb[1][:, kc, :], xt_d.ap()[1, kc])
                    b2_full = cpool.tile([P, D], f32)
                    nc.sync.dma_start(
                        b2_full[:],
                        b2_d.ap()[None, :].broadcast_to([P, D]))
                if hc == 5:
                    for kc in range(KC):
                        nc.sync.dma_start(x_sb[2][:, kc, :], xt_d.ap()[2, kc])

            def layer1_pair(g, j):
                """h^T for h-chunks (2j, 2j+1): DoubleRow matmuls + relu
                into a [P, 2, GROUP] fp8 pair tile."""
                hsb2 = hpool.tile([P, 2, GROUP], fp8)
                for i in range(2):
                    hc = 2 * j + i
                    hps = hpsum.tile([P, GROUP], f32)
                    for k2 in range(KC // 2):
                        nc.tensor.matmul(
                            hps[:],
                            w1_all[:, hc, 2 * k2:2 * k2 + 2, :],
                            x_sb[g][:, 2 * k2:2 * k2 + 2, :],
                            start=(k2 == 0), stop=(k2 == KC // 2 - 1),
                            perf_mode=DR,
                        )
                    # relu(acc/(SX*S1) + b1) * SH, written as
                    # relu(acc * SH/(SX*S1) + SH*b1)  (b1 pre-scaled on host)
                    nc.scalar.activation(
                        hsb2[:, i, :], hps[:], AF.Relu,
                        bias=b1_sb[:, hc:hc + 1], scale=SH / (SX * S1))
                return hsb2

            def layer2_pair(g, j, hsb2, yps, last):
                for m in range(MSUB):
                    lhs = hsb2[:, :, m * P:(m + 1) * P]
                    for h2 in range(2):
                        nc.tensor.matmul(
                            yps[m][:, h2 * 512:(h2 + 1) * 512],
                            lhs,
                            w2_all[:, 2 * j:2 * j + 2,
                                   h2 * 512:(h2 + 1) * 512],
                            start=(j == 0 and not last),
                            stop=(j == HPAIR - 1),
                            perf_mode=DR,
                        )

            for g in range(NGRP):
                yps = [ypsum.tile([P, D], f32, name=f"yps{m}", tag=f"yps{m}")
                       for m in range(MSUB)]

                last = g == NGRP - 1
                if last:
                    # rank-1 b2 matmuls keep the last group's tail short
                    # (no DVE add on the critical path)
                    for m in range(MSUB):
                        for h2 in range(2):
                            nc.tensor.matmul(
                                yps[m][:, h2 * 512:(h2 + 1) * 512],
                                ones_sb[:],
                                b2h_sb[:, h2 * 512:(h2 + 1) * 512],
                                start=True, stop=False,
                            )

                # Software pipeline: issue layer-1 for pair j+2 before
                # layer-2 of pair j so the PE never waits on the relu.
                hq = [layer1_pair(g, 0), layer1_pair(g, 1)]
                for j in range(HPAIR):
                    if j + 2 < HPAIR:
                        hq.append(layer1_pair(g, j + 2))
                    layer2_pair(g, j, hq[j], yps, last)

                # Epilogue: (+ b2 via DVE unless folded), sigmoid, store
                for m in range(MSUB):
                    if not last:
                        nc.vector.tensor_add(yps[m][:], yps[m][:], b2_full[:])
                    yo = ypool_sb.tile([P, D], f32)
                    nc.scalar.activation(yo[:], yps[m][:], AF.Sigmoid,
                                         scale=1.0 / (SH * S2))
                    nc.sync.dma_start(y_v[g, m], yo[:])

    nc.compile()
    return nc


def _routing(x, Wr, br):
    """Router computed with the same jax expression as the reference."""
    import jax
    import jax.numpy as jnp

    logits = jnp.asarray(x) @ jnp.asarray(Wr) + jnp.asarray(br)
    topk_vals, topk_idx = jax.lax.top_k(logits, TOPK)
    weights = jax.nn.softmax(topk_vals, axis=-1)
    return np.asarray(topk_idx), np.asarray(weights, np.float32)


def _get_nc():
    global _compiled_nc
    if _compiled_nc is None:
        _compiled_nc = _build_nc()
    return _compiled_nc


def _to_fp8(a):
    import ml_dtypes
    return a.astype(ml_dtypes.float8_e4m3fn)


def kernel(x, Wr, br, W1, b1, W2, b2, _trace=False, _trace_kwargs=None):
    from concourse import bass_utils

    x = np.ascontiguousarray(np.asarray(x, dtype=np.float32))
    Wr = np.asarray(Wr, dtype=np.float32)
    br = np.asarray(br, dtype=np.float32)
    W1 = np.asarray(W1, dtype=np.float32)
    b1 = np.asarray(b1, dtype=np.float32)
    W2 = np.asarray(W2, dtype=np.float32)
    b2 = np.asarray(b2, dtype=np.float32)

    topk_idx, wts = _routing(x, Wr, br)

    # Per-expert token lists and weights
    tok_lists = []
    wt_lists = []
    for e in range(E):
        mask = topk_idx == e                      # [B, TOPK]
        toks = np.nonzero(mask.any(axis=1))[0]
        # weight of expert e for each selected token (exactly one slot matches)
        slot = mask[toks].argmax(axis=1)
        tok_lists.append(toks)
        wt_lists.append(wts[toks, slot])

    nc = _get_nc()

    xq = _to_fp8(x * SX)
    # chunk-major w1 layout: [HC, P, KC, P], scaled by S1
    W1ch = [np.ascontiguousarray(
        _to_fp8(W1[e] * S1).reshape(KC, P, HC, P).transpose(2, 1, 0, 3))
        for e in range(E)]
    W2q = [np.ascontiguousarray(_to_fp8(W2[e] * S2)) for e in range(E)]

    out = np.zeros((B, D), dtype=np.float32)
    max_count = max(len(t) for t in tok_lists)
    n_waves = max(1, -(-max_count // CAP))
    last_result = None
    for wave in range(n_waves):
        in_maps = []
        for e in range(E):
            toks = tok_lists[e][wave * CAP:(wave + 1) * CAP]
            xpad = np.zeros((CAP, D), dtype=xq.dtype)
            if len(toks):
                xpad[:len(toks)] = xq[toks]
            xt = np.ascontiguousarray(
                xpad.reshape(NGRP, GROUP, KC, P).transpose(0, 2, 3, 1))
            in_maps.append({
                "xt": xt,
                "ones": np.ones((P,), dtype=np.float16),
                "b2h": (b2[e] * SH * S2).astype(np.float16),
                "w1": W1ch[e],
                "b1": np.ascontiguousarray(b1[e] * SH),
                "w2": W2q[e],
                "b2": np.ascontiguousarray(b2[e] * SH * S2),
            })
        res = bass_utils.run_bass_kernel_spmd(
            nc, in_maps, core_ids=list(range(N_CORES)),
            trace=_trace, **(_trace_kwargs or {}))
        last_result = res
        for e in range(E):
            toks = tok_lists[e][wave * CAP:(wave + 1) * CAP]
            if len(toks) == 0:
                continue
            y_e = res.results[e]["y"][:len(toks)]
            out[toks] += wt_lists[e][wave * CAP:(wave + 1) * CAP][:, None] * y_e

    if _trace:
        kernel.last_result = last_result
    return out


# revision 3
# speedup vs baseline: 2.1781x; 1.0927x over previous
"""MoE block (router + top-2 expert MLPs) on 8 Trainium2 NeuronCores.

Strategy (expert-parallel, fp8 DoubleRow):
  - Router (x @ Wr + br, top-2, softmax) computed on host with jax using the
    exact expression of the reference so expert selection matches bitwise.
  - Tokens are dispatched by expert: core e receives the tokens whose top-2
    includes expert e (padded to a fixed capacity CAP), plus expert e's
    weights W1[e]/b1[e]/W2[e]/b2[e].
  - Each core runs a Bass/Tile kernel computing
        y = sigmoid(relu(x @ W1 + b1) @ W2 + b2)
    for its CAP tokens with fp8-e4m3 matmuls in DoubleRow perf mode
    (2 fp8 weights per PE cell -> K=256 contraction per pass, ~1.4-1.8x
    the fp16 matmul throughput). fp32 PSUM accumulation.
  - Quantization scales (powers of 2, exact to undo): x*16, W1*2048,
    h*32, W2*4096. relu is positively homogeneous so the h scale folds
    into the layer-1 activation (scale=2^-10 on PSUM, bias=32*b1);
    the final sigmoid applies scale=2^-17 to undo h/W2 scaling. b2 is
    pre-scaled by 2^17 on host (fp32/fp16, exact enough).
  - Host combines: out[t] = sum_k weight[t,k] * y_e[t].

Kernel layout per core:
  xT [NGRP, 128, KC, GROUP] fp8 (tokens gathered+transposed+scaled on
  host; one 3 KiB/partition DMA per group),
  W1 [HPAIR, 128, 2, KC, 128] fp8 and W2 [HPAIR, 128, 2, D] fp8 (one
  2 KiB/partition DMA per h-chunk pair; ~250 GB/s sustained),
  b1 fp32 (*32), b2 fp32 (*2^17). All weights are SBUF-resident; they
  stream from HBM exactly once, deadline-ordered: x(g0), first w1
  pairs, then rounds of two w1 pairs + one w2 pair (layer 1 consumes
  w1 about twice as fast as layer 2 consumes w2), w2 tail with
  x(g1)/x(g2)/b2 interleaved.
  Loop over 3 token groups of 384; per group y accumulates in PSUM
  (3 x [128 tok, 1024 d] fp32 tiles = 6 banks) across 16 h-chunk PAIRS;
  the h PSUM tiles (128 h x 384 tok, 2 banks) double-buffer.
  Layer 1 (per h-chunk): 4 DoubleRow matmuls lhsT=W1[:, kc:kc+2, :],
  rhs=xT[:, kc:kc+2, :] -> h^T in PSUM; relu+b1 via ScalarE into a
  [128, 2, GROUP] fp8 pair tile; layer 2: lhsT=pair tile slice
  [128, 2, 128 tok], rhs=W2 pair [128, 2, 512], DoubleRow, accumulating
  into the y PSUM tiles. The layer-1 work for pair j+2 is issued before
  layer-2 of pair j so the PE never stalls on the relu latency.
  Epilogue runs at 512-column half granularity (DVE b2-add, ScalarE
  sigmoid -> fp16, DMA out) so the next group's first accumulation
  (WAR on the y PSUM banks) unblocks as early as possible. b2 is
  pre-added via rank-1 (K=1) fp16 matmuls on the last group to keep
  the final tail off the DVE.
"""

import numpy as np

D = 1024
H = 4096
E = 8
TOPK = 2
B = 4096

P = 128
KC = D // P          # 8 contraction chunks for layer 1
HC = H // P          # 32 h chunks
HPAIR = HC // 2      # 16 h-chunk pairs (DoubleRow)
GROUP = 384          # tokens per PSUM-resident group
MSUB = GROUP // P    # 3 token subtiles per group
NGRP = 3             # groups per core
CAP = GROUP * NGRP   # 1152 token capacity per core
N_CORES = 8

# fp8 quantization scales (powers of two; exactly undone on device)
SX = 16.0
S1 = 2048.0
SH = 32.0
S2 = 4096.0

_compiled_nc = None


def _build_nc():
    import concourse.bacc as bacc
    import concourse.mybir as mybir
    import concourse.tile as tile

    f32 = mybir.dt.float32
    f16 = mybir.dt.float16
    fp8 = mybir.dt.float8e4
    AF = mybir.ActivationFunctionType
    DR = mybir.MatmulPerfMode.DoubleRow

    nc = bacc.Bacc("TRN2", target_bir_lowering=False, debug=False,
                   enable_asserts=False)

    # Host-prearranged layouts: every chunk is one contiguous DMA.
    #   xt[g, p, kc, t'] = SX * x_tokens[g*GROUP + t', kc*128 + p]
    #   w1[j, p, i, kc, h'] = S1 * W1[kc*128 + p, (2j+i)*128 + h']
    #   w2[j, p, i, d] = S2 * W2[(2j+i)*128 + p, d]
    xt_d = nc.dram_tensor("xt", (NGRP, P, KC, GROUP), fp8,
                          kind="ExternalInput")
    w1_d = nc.dram_tensor("w1", (HPAIR, P, 2, KC, P), fp8,
                          kind="ExternalInput")
    b1_d = nc.dram_tensor("b1", (H,), f32, kind="ExternalInput")  # *SH
    w2_d = nc.dram_tensor("w2", (HPAIR, P, 2, D), fp8,
                          kind="ExternalInput")  # *S2
    b2_d = nc.dram_tensor("b2", (D,), f32, kind="ExternalInput")  # *SH*S2
    b2h_d = nc.dram_tensor("b2h", (D,), f16, kind="ExternalInput")  # *SH*S2
    ones_d = nc.dram_tensor("ones", (P,), f16, kind="ExternalInput")
    y_d = nc.dram_tensor("y", (CAP, D), f16, kind="ExternalOutput")

    b1_v = b1_d.ap().rearrange("(hc p) -> p hc", p=P)          # [128, 32]
    y_v = y_d.ap().rearrange("(g m p) d -> g m p d", g=NGRP, m=MSUB)

    with tile.TileContext(nc) as tc:
        with (
            tc.tile_pool(name="const", bufs=1) as cpool,
            tc.tile_pool(name="wres", bufs=1) as respool,
            tc.tile_pool(name="hsb", bufs=3) as hpool,
            tc.tile_pool(name="yout", bufs=4) as ypool_sb,
            tc.tile_pool(name="hps", bufs=2, space="PSUM") as hpsum,
            tc.tile_pool(name="yps", bufs=1, space="PSUM") as ypsum,
        ):
            x_sb = [cpool.tile([P, KC, GROUP], fp8, name=f"x{g}",
                               tag=f"x{g}") for g in range(NGRP)]
            # PE warm-up: dependency-free matmuls on an uninitialized
            # scratch tile get the PE past the HAM half-clock window while
            # the first input DMAs are still in flight. Results land in a
            # scratch PSUM tile and are never read.
            scratch_sb = cpool.tile([P, GROUP], fp8)
            nc.vector.memset(scratch_sb[:], 0.0)
            warm_ps = hpsum.tile([P, GROUP], f32, name="warm_ps", tag="hps")
            for _ in range(12):
                nc.tensor.matmul(warm_ps[:], scratch_sb[:, :P],
                                 scratch_sb[:], start=True, stop=True)

            w1_all = respool.tile([P, HC, KC, P], fp8)
            w2_all = respool.tile([P, HC, D], fp8)

            def dma_w1(j):
                # [P, 2, KC, P] pair chunk -> w1_all[:, 2j:2j+2]
                nc.sync.dma_start(w1_all[:, 2 * j:2 * j + 2], w1_d.ap()[j])

            def dma_w2(j):
                nc.sync.dma_start(w2_all[:, 2 * j:2 * j + 2, :], w2_d.ap()[j])

            # Deadline-ordered input stream.
            nc.sync.dma_start(x_sb[0][:], xt_d.ap()[0])
            dma_w1(0)
            dma_w1(1)
            b1_sb = cpool.tile([P, HC], f32)
            nc.sync.dma_start(b1_sb[:], b1_v)
            b2h_sb = cpool.tile([1, D], f16)
            nc.sync.dma_start(b2h_sb[:], b2h_d.ap()[None, :])
            ones_sb = cpool.tile([1, P], f16)
            nc.sync.dma_start(ones_sb[:], ones_d.ap()[None, :])
            w2_head = 0
            for k in range(7):
                dma_w2(w2_head)
                w2_head += 1
                dma_w1(2 + 2 * k)
                dma_w1(3 + 2 * k)
            b2_full = None
            for j in range(w2_head, HPAIR):
                dma_w2(j)
                if j == 9:
                    nc.sync.dma_start(x_sb[1][:], xt_d.ap()[1])
                if j == 11:
                    b2_full = cpool.tile([P, D], f32)
                    nc.sync.dma_start(
                        b2_full[:],
                        b2_d.ap()[None, :].broadcast_to([P, D]))
                if j == 13:
                    nc.sync.dma_start(x_sb[2][:], xt_d.ap()[2])

            def layer1_pair(g, j):
                """h^T for h-chunks (2j, 2j+1): DoubleRow matmuls + relu
                into a [P, 2, GROUP] fp8 pair tile."""
                hsb2 = hpool.tile([P, 2, GROUP], fp8)
                for i in range(2):
                    hc = 2 * j + i
                    hps = hpsum.tile([P, GROUP], f32)
                    for k2 in range(KC // 2):
                        nc.tensor.matmul(
                            hps[:],
                            w1_all[:, hc, 2 * k2:2 * k2 + 2, :],
                            x_sb[g][:, 2 * k2:2 * k2 + 2, :],
                            start=(k2 == 0), stop=(k2 == KC // 2 - 1),
                            perf_mode=DR,
                        )
                    # relu(acc/(SX*S1) + b1) * SH, written as
                    # relu(acc * SH/(SX*S1) + SH*b1)  (b1 pre-scaled on host)
                    nc.scalar.activation(
                        hsb2[:, i, :], hps[:], AF.Relu,
                        bias=b1_sb[:, hc:hc + 1], scale=SH / (SX * S1))
                return hsb2

            def layer2_pair(g, j, hsb2, yps, last):
                for m in range(MSUB):
                    lhs = hsb2[:, :, m * P:(m + 1) * P]
                    for h2 in range(2):
                        nc.tensor.matmul(
                            yps[m][:, h2 * 512:(h2 + 1) * 512],
                            lhs,
                            w2_all[:, 2 * j:2 * j + 2,
                                   h2 * 512:(h2 + 1) * 512],
                            start=(j == 0 and not last),
                            stop=(j == HPAIR - 1),
                            perf_mode=DR,
                        )

            for g in range(NGRP):
                yps = [ypsum.tile([P, D], f32, name=f"yps{m}", tag=f"yps{m}")
                       for m in range(MSUB)]

                last = g == NGRP - 1

                # Software pipeline: issue layer-1 for pair j+2 before
                # layer-2 of pair j so the PE never waits on the relu.
                hq = [layer1_pair(g, 0)]
                if last:
                    # rank-1 b2 matmuls keep the last group's tail short
                    # (no DVE add on the critical path)
                    for m in range(MSUB):
                        for h2 in range(2):
                            nc.tensor.matmul(
                                yps[m][:, h2 * 512:(h2 + 1) * 512],
                                ones_sb[:],
                                b2h_sb[:, h2 * 512:(h2 + 1) * 512],
                                start=True, stop=False,
                            )
                hq.append(layer1_pair(g, 1))
                for j in range(HPAIR):
                    if j + 2 < HPAIR:
                        hq.append(layer1_pair(g, j + 2))
                    layer2_pair(g, j, hq[j], yps, last)

                # Epilogue at 512-column halves: (+ b2 via DVE unless
                # folded), sigmoid -> fp16, store. Finer granularity lets
                # the next group's PSUM reuse (WAR) clear sooner.
                for m in range(MSUB):
                    for h2 in range(2):
                        sl = slice(h2 * 512, (h2 + 1) * 512)
                        if not last:
                            nc.vector.tensor_add(yps[m][:, sl], yps[m][:, sl],
                                                 b2_full[:, sl])
                        yo = ypool_sb.tile([P, 512], f16)
                        nc.scalar.activation(yo[:], yps[m][:, sl], AF.Sigmoid,
                                             scale=1.0 / (SH * S2))
                        nc.sync.dma_start(y_v[g, m][:, sl], yo[:])

    nc.compile()
    return nc


def _routing(x, Wr, br):
    """Router computed with the same jax expression as the reference."""
    import jax
    import jax.numpy as jnp

    logits = jnp.asarray(x) @ jnp.asarray(Wr) + jnp.asarray(br)
    topk_vals, topk_idx = jax.lax.top_k(logits, TOPK)
    weights = jax.nn.softmax(topk_vals, axis=-1)
    return np.asarray(topk_idx), np.asarray(weights, np.float32)


def _get_nc():
    global _compiled_nc
    if _compiled_nc is None:
        _compiled_nc = _build_nc()
    return _compiled_nc


def _to_fp8(a):
    import ml_dtypes
    return a.astype(ml_dtypes.float8_e4m3fn)


def kernel(x, Wr, br, W1, b1, W2, b2, _trace=False, _trace_kwargs=None):
    from concourse import bass_utils

    x = np.ascontiguousarray(np.asarray(x, dtype=np.float32))
    Wr = np.asarray(Wr, dtype=np.float32)
    br = np.asarray(br, dtype=np.float32)
    W1 = np.asarray(W1, dtype=np.float32)
    b1 = np.asarray(b1, dtype=np.float32)
    W2 = np.asarray(W2, dtype=np.float32)
    b2 = np.asarray(b2, dtype=np.float32)

    topk_idx, wts = _routing(x, Wr, br)

    # Per-expert token lists and weights
    tok_lists = []
    wt_lists = []
    for e in range(E):
        mask = topk_idx == e                      # [B, TOPK]
        toks = np.nonzero(mask.any(axis=1))[0]
        # weight of expert e for each selected token (exactly one slot matches)
        slot = mask[toks].argmax(axis=1)
        tok_lists.append(toks)
        wt_lists.append(wts[toks, slot])

    nc = _get_nc()

    xq = _to_fp8(x * SX)
    # pair-chunk w1 layout: [HPAIR, P, 2, KC, P], scaled by S1
    W1ch = [np.ascontiguousarray(
        _to_fp8(W1[e] * S1).reshape(KC, P, HPAIR, 2, P)
        .transpose(2, 1, 3, 0, 4)) for e in range(E)]
    # pair-chunk w2 layout: [HPAIR, P, 2, D], scaled by S2
    W2ch = [np.ascontiguousarray(
        _to_fp8(W2[e] * S2).reshape(HPAIR, 2, P, D).transpose(0, 2, 1, 3))
        for e in range(E)]

    out = np.zeros((B, D), dtype=np.float32)
    max_count = max(len(t) for t in tok_lists)
    n_waves = max(1, -(-max_count // CAP))
    last_result = None
    for wave in range(n_waves):
        in_maps = []
        for e in range(E):
            toks = tok_lists[e][wave * CAP:(wave + 1) * CAP]
            xpad = np.zeros((CAP, D), dtype=xq.dtype)
            if len(toks):
                xpad[:len(toks)] = xq[toks]
            # [NGRP, P, KC, GROUP]: xt[g, p, kc, t] = xpad[g*384+t, kc*128+p]
            xt = np.ascontiguousarray(
                xpad.reshape(NGRP, GROUP, KC, P).transpose(0, 3, 2, 1))
            in_maps.append({
                "xt": xt,
                "ones": np.ones((P,), dtype=np.float16),
                "b2h": (b2[e] * SH * S2).astype(np.float16),
                "w1": W1ch[e],
                "b1": np.ascontiguousarray(b1[e] * SH),
                "w2": W2ch[e],
                "b2": np.ascontiguousarray(b2[e] * SH * S2),
            })
        res = bass_utils.run_bass_kernel_spmd(
            nc, in_maps, core_ids=list(range(N_CORES)),
            trace=_trace, **(_trace_kwargs or {}))
        last_result = res
        for e in range(E):
            toks = tok_lists[e][wave * CAP:(wave + 1) * CAP]
            if len(toks) == 0:
                continue
            y_e = res.results[e]["y"][:len(toks)].astype(np.float32)
            out[toks] += wt_lists[e][wave * CAP:(wave + 1) * CAP][:, None] * y_e

    if _trace:
        kernel.last_result = last_result
    return out
